# revision 9
# baseline (speedup 1.0000x reference)
"""Trainium2 Bass kernel: 32-bit soft-logic comparator (A > B, A == B).

Inputs A, B: [2_000_000, 32] float32 with values in {0.0, 1.0}, MSB first.
Outputs: (a_gt_b, a_eq_b), each [2_000_000, 1] float32 in {0.0, 1.0}.

Algorithm (exact in fp32, replaces the reference's prefix-product ladder):
  d_i = a_i - b_i in {-1, 0, 1}
  S_hi = sum_{i<16} d_i * 2^(31-i)   (integer multiple of 2^16, |.| < 2^32,
                                      every partial sum has <=16-bit mantissa
                                      => exact in fp32 in any order)
  S_lo = sum_{i>=16} d_i * 2^(31-i)  (integer, |.| <= 65535 => exact)
  V    = S_hi + S_lo                 (one correctly-rounded add: sign exact,
                                      V == 0 iff S_hi == S_lo == 0)
  a_gt_b = (V > 0), a_eq_b = (V == 0)

The device clamps V to {-1,0,1} (exact: V is integer-valued and sign-exact)
and stores a single int8 map in partition-major layout with ONE line-rate DMA
per pass; the host unpermutes and decodes the two boolean maps. That cuts
output HBM traffic 8x vs storing two fp32 maps and keeps the HWDGE input
queues free of small store descriptors.

Sharding: data parallel along dim 0 across 8 cores. Rows per core padded to
a multiple of 128 (250112 = 128 * 1954); only the last core's shard needs
host-side zero padding (896 rows), and the pad rows are dropped on gather.

Input layout options (host-side, same bytes and dtype either way):
  sep: A and B as separate DRAM tensors, streamed on the two HWDGE rings.
  ab:  A and B tiles interleaved into one DRAM tensor so HBM sees a single
       sequential address stream; one DMA per tile, alternating rings.
"""

import numpy as np

N = 2_000_000
BITS = 32
NCORES = 8
P = 128
ROWS_PER_CORE = 250_112          # 128 * 1954 >= 2_000_000 / 8
GROUPS = ROWS_PER_CORE // P      # 1954 rows per partition per core
K_MAIN = 128
KS = [K_MAIN] * (GROUPS // K_MAIN) + ([GROUPS % K_MAIN] if GROUPS % K_MAIN else [])
SEG = 16                         # bits per exact-sum segment
IO_BUFS = 4
LAYOUT = "sep"                   # "sep" or "ab"
STORE = "pass"                   # "tile": per-tile stores; "pass": one store/pass
OUT_INT8 = True                  # with STORE="pass": clamp V to {-1,0,1} int8
INPUT_CAST = True                # SWDGE fp32->bf16 cast during input DMA

_CACHE = {}


def _weight_row():
    # w_i = 2^(31-i), MSB first; exact in fp32 and bf16 (powers of two).
    return (2.0 ** (31 - np.arange(BITS, dtype=np.float64))).astype(np.float32)


def _compute_tail(nc, pool, spool, wt, ov_flat, mybir, row0, k, d,
                  vb=None, g0=0):
    """Shared mult/reduce/add/store chain for one tile, given d = a - b."""
    dt = mybir.dt
    Alu = mybir.AluOpType
    Axis = mybir.AxisListType
    rows = P * k
    F = k * BITS

    nc.vector.tensor_tensor(d[:], d[:], wt[:, :F], Alu.mult)

    # Segmented sums of 16 -> [P, 2k] (hi, lo interleaved per row);
    # accumulation is fp32 (out dtype), every addend exact.
    s = spool.tile([P, 2 * k], dt.float32, tag="s")
    nc.vector.tensor_reduce(
        out=s[:],
        in_=d[:].rearrange("p (g x) -> p g x", x=SEG),
        axis=Axis.X,
        op=Alu.add,
    )

    s3 = s[:].rearrange("p (r two) -> p r two", two=2)
    if vb is None:
        # V = S_hi + S_lo, emitted directly as bf16 for a compact store.
        v = spool.tile([P, k], dt.bfloat16, tag="v")
        nc.vector.tensor_tensor(v[:], s3[:, :, 0:1], s3[:, :, 1:2], Alu.add)
        # Tiny store goes on gpsimd SWDGE, keeping HWDGE queues for inputs.
        nc.gpsimd.dma_start(
            out=ov_flat[row0:row0 + rows].rearrange("(p r) -> p r", p=P),
            in_=v[:])
    elif OUT_INT8:
        # Clamp V to {-1,0,1} (exact: V is integer-valued) and pack int8
        # into the pass-level buffer; one line-rate store per pass.
        v32 = spool.tile([P, k], dt.float32, tag="v32")
        nc.vector.tensor_tensor(v32[:], s3[:, :, 0:1], s3[:, :, 1:2], Alu.add)
        nc.vector.tensor_scalar(v32[:], v32[:], 1.0, None, Alu.min)
        nc.vector.tensor_scalar(vb[:, g0:g0 + k], v32[:], -1.0, None, Alu.max)
    else:
        nc.vector.tensor_tensor(vb[:, g0:g0 + k], s3[:, :, 0:1], s3[:, :, 1:2],
                                Alu.add)


def _emit_pass_sep(nc, pool, spool, wt, a_flat, b_flat, ov_flat, mybir,
                   dma_only=False, vpool=None, ov2d=None):
    dt = mybir.dt
    Alu = mybir.AluOpType
    vb = None
    if STORE == "pass" and not dma_only:
        odt = dt.int8 if OUT_INT8 else dt.bfloat16
        vb = vpool.tile([P, GROUPS], odt, tag="vb")
    row0 = 0
    g0 = 0
    in_dt = dt.bfloat16 if INPUT_CAST else dt.float32
    for k in KS:
        rows = P * k
        F = k * BITS
        a = pool.tile([P, F], in_dt, tag="a")
        b = pool.tile([P, F], in_dt, tag="b")
        av = a_flat[row0 * BITS:(row0 + rows) * BITS].rearrange("(p f) -> p f", p=P)
        bv = b_flat[row0 * BITS:(row0 + rows) * BITS].rearrange("(p f) -> p f", p=P)
        if INPUT_CAST:
            # SWDGE casts fp32->bf16 in the SDMA datapath: HBM still reads
            # the full fp32 stream, but only half the bytes cross the SBUF
            # AXI ports ({0,1} is exact in bf16).
            nc.gpsimd.dma_start(out=a[:], in_=av)
            nc.gpsimd.dma_start(out=b[:], in_=bv)
        else:
            # Split input streaming across both HWDGE issuing engines.
            nc.sync.dma_start(out=a[:], in_=av)
            nc.scalar.dma_start(out=b[:], in_=bv)
        if not dma_only:
            # d <- (a - b) in bf16 (exact: values in {-1,0,1}); frees a/b
            # after one op, and bf16 runs the mult/reduce at 2x DVE rate.
            d = pool.tile([P, F], dt.bfloat16, tag="d")
            nc.vector.tensor_tensor(d[:], a[:], b[:], Alu.subtract)
            _compute_tail(nc, pool, spool, wt, ov_flat, mybir, row0, k, d,
                          vb=vb, g0=g0)
        row0 += rows
        g0 += k
    assert row0 == ROWS_PER_CORE
    if vb is not None:
        nc.gpsimd.dma_start(out=ov2d[:, :], in_=vb[:])


def _emit_pass_ab(nc, pool, spool, wt, ab_flat, ov_flat, mybir,
                  dma_only=False):
    dt = mybir.dt
    Alu = mybir.AluOpType
    row0 = 0
    for ti, k in enumerate(KS):
        rows = P * k
        F = k * BITS
        t = pool.tile([P, 2 * F], dt.float32, tag="t")
        tv = ab_flat[row0 * 2 * BITS:(row0 + rows) * 2 * BITS] \
            .rearrange("(p f) -> p f", p=P)
        # One sequential 2F-wide stream per tile; alternate HWDGE rings.
        eng = nc.sync if ti % 2 == 0 else nc.scalar
        eng.dma_start(out=t[:], in_=tv)
        if not dma_only:
            d = pool.tile([P, F], dt.bfloat16, tag="d")
            nc.vector.tensor_tensor(d[:], t[:, :F], t[:, F:], Alu.subtract)
            _compute_tail(nc, pool, spool, wt, ov_flat, mybir, row0, k, d)
        row0 += rows
    assert row0 == ROWS_PER_CORE


def _legalize_waits(nc, mybir):
    """TRN2 ISA structs accept at most one sync wait per instruction (walrus
    codegen hard-errors otherwise). Tile's scheduler attaches one wait per
    dependency, so hoist all-but-one wait onto same-engine NoOps inserted
    immediately before; engines execute in order, so semantics are identical."""
    for fn in nc.m.functions:
        for blk in fn.blocks:
            new_insts = []
            for inst in blk.instructions:
                si = inst.sync_info
                waits = list(si.on_wait) if si is not None else []
                limit = 2 if isinstance(inst, mybir.InstEventSemaphore) else 1
                if len(waits) > limit:
                    for w in waits[:-limit]:
                        nop = mybir.InstNoOp(
                            name=nc.get_next_instruction_name(),
                            sync_info=mybir.SyncInfo(on_wait=[w], on_update=[]),
                            bass_nofuse=True,
                            engine=inst.engine,
                        )
                        nc.register_instruction(nop)
                        new_insts.append(nop)
                    si.on_wait = waits[-limit:]
                new_insts.append(inst)
            blk.instructions[:] = new_insts


def _build_program(repeat=1, dma_only=False):
    key = ("nc", repeat, dma_only, LAYOUT, K_MAIN, IO_BUFS, tuple(KS),
           STORE, OUT_INT8, INPUT_CAST)
    if key in _CACHE:
        return _CACHE[key]

    from concourse.bass import Bass
    from concourse.tile import TileContext
    import concourse.mybir as mybir

    dt = mybir.dt

    nc = Bass(name="cmp32")
    if LAYOUT == "sep":
        A = nc.dram_tensor("A", [ROWS_PER_CORE, BITS], dt.float32,
                           kind="ExternalInput")
        B = nc.dram_tensor("B", [ROWS_PER_CORE, BITS], dt.float32,
                           kind="ExternalInput")
        in_flats = (A[:].flatten(), B[:].flatten())
    else:
        AB = nc.dram_tensor("AB", [2 * ROWS_PER_CORE, BITS], dt.float32,
                            kind="ExternalInput")
        in_flats = (AB[:].flatten(),)
    W = nc.dram_tensor("W", [P, K_MAIN * BITS], dt.bfloat16, kind="ExternalInput")
    if STORE == "pass":
        odt = dt.int8 if OUT_INT8 else dt.bfloat16
        OV = nc.dram_tensor("OV", [P, GROUPS], odt, kind="ExternalOutput")
        ov_flat, ov2d = None, OV
    else:
        OV = nc.dram_tensor("OV", [ROWS_PER_CORE, 1], dt.bfloat16,
                            kind="ExternalOutput")
        ov_flat, ov2d = OV[:].flatten(), None

    with TileContext(nc) as tc:
        with tc.tile_pool(name="wpool", bufs=1) as wpool, \
             tc.tile_pool(name="io", bufs=IO_BUFS) as pool, \
             tc.tile_pool(name="small", bufs=4) as spool, \
             tc.tile_pool(name="vpass", bufs=2) as vpool:
            # bf16 weight tile (weights are powers of two: exact in bf16)
            wt = wpool.tile([P, K_MAIN * BITS], dt.bfloat16)
            nc.gpsimd.dma_start(out=wt[:], in_=W[:])

            for _rep in range(repeat):
                if LAYOUT == "sep":
                    _emit_pass_sep(nc, pool, spool, wt, *in_flats, ov_flat,
                                   mybir, dma_only=dma_only,
                                   vpool=vpool, ov2d=ov2d)
                else:
                    _emit_pass_ab(nc, pool, spool, wt, *in_flats, ov_flat,
                                  mybir, dma_only=dma_only)

    _legalize_waits(nc, mybir)
    _CACHE[key] = nc
    return nc


def _interleave_ab(a_sh, b_sh):
    """Per-core: tile-interleaved single stream. For each tile of k row-groups,
    partition p's k a-rows are followed by its k b-rows; tiles are sequential.
    Same bytes and dtype as the inputs, only the address layout changes."""
    out = np.empty((2 * ROWS_PER_CORE, BITS), dtype=np.float32)
    o = 0
    row0 = 0
    for k in KS:
        rows = P * k
        blk_a = a_sh[row0:row0 + rows].reshape(P, k * BITS)
        blk_b = b_sh[row0:row0 + rows].reshape(P, k * BITS)
        blk = np.concatenate([blk_a, blk_b], axis=1)   # [P, 2F]
        out[o:o + 2 * rows] = blk.reshape(2 * rows, BITS)
        o += 2 * rows
        row0 += rows
    return out


def _shard_inputs(A, B):
    """Split full inputs into 8 per-core maps (zero-pad only the last core)."""
    import ml_dtypes
    w_tile = np.tile(_weight_row(), (P, K_MAIN)).astype(ml_dtypes.bfloat16)
    total = ROWS_PER_CORE * NCORES
    pad = total - N
    in_maps = []
    for c in range(NCORES):
        lo, hi = c * ROWS_PER_CORE, (c + 1) * ROWS_PER_CORE
        if hi <= N:
            a_sh, b_sh = A[lo:hi], B[lo:hi]
        else:
            z = np.zeros((pad, BITS), dtype=np.float32)
            a_sh = np.concatenate([A[lo:N], z])
            b_sh = np.concatenate([B[lo:N], z])
        if LAYOUT == "sep":
            in_maps.append({"A": a_sh, "B": b_sh, "W": w_tile})
        else:
            in_maps.append({"AB": _interleave_ab(a_sh, b_sh), "W": w_tile})
    return in_maps


def _decode(v):
    """Device output V (bf16) -> the two fp32 boolean maps."""
    v32 = np.asarray(v).astype(np.float32)
    og = (v32 > 0).astype(np.float32)
    oe = (v32 == 0).astype(np.float32)
    return og, oe


def _core_v_from_pass(arr):
    """Unpermute one core's [P, GROUPS] pass-store output to row order."""
    out = np.empty((ROWS_PER_CORE,), dtype=np.float32)
    g0 = row0 = 0
    for k in KS:
        out[row0:row0 + P * k] = arr[:, g0:g0 + k].astype(np.float32).reshape(-1)
        g0 += k
        row0 += P * k
    return out


def _postprocess(core_arrs):
    """Per-core device outputs -> (a_gt_b, a_eq_b) full fp32 maps."""
    if STORE == "pass":
        v32 = np.concatenate([_core_v_from_pass(np.asarray(a))
                              for a in core_arrs])[:N].reshape(-1, 1)
        og = (v32 > 0).astype(np.float32)
        oe = (v32 == 0).astype(np.float32)
        return og, oe
    v = np.concatenate([np.asarray(a) for a in core_arrs])[:N]
    return _decode(v)


def kernel(A, B):
    from concourse.bass_utils import run_bass_kernel_spmd

    A = np.ascontiguousarray(A, dtype=np.float32)
    B = np.ascontiguousarray(B, dtype=np.float32)
    assert A.shape == (N, BITS) and B.shape == (N, BITS)

    nc = _build_program()
    in_maps = _shard_inputs(A, B)
    res = run_bass_kernel_spmd(nc, in_maps, core_ids=list(range(NCORES)))
    og, oe = _postprocess([r["OV"] for r in res.results])
    return og, oe


# revision 10
# speedup vs baseline: 1.0000x; 1.0000x over previous
"""Trainium2 Bass kernel: 32-bit soft-logic comparator (A > B, A == B).

Inputs A, B: [2_000_000, 32] float32 with values in {0.0, 1.0}, MSB first.
Outputs: (a_gt_b, a_eq_b), each [2_000_000, 1] float32 in {0.0, 1.0}.

Algorithm (exact in fp32, replaces the reference's prefix-product ladder):
  d_i = a_i - b_i in {-1, 0, 1}
  S_hi = sum_{i<16} d_i * 2^(31-i)   (integer multiple of 2^16, |.| < 2^32,
                                      every partial sum has <=16-bit mantissa
                                      => exact in fp32 in any order)
  S_lo = sum_{i>=16} d_i * 2^(31-i)  (integer, |.| <= 65535 => exact)
  V    = S_hi + S_lo                 (one correctly-rounded add: sign exact,
                                      V == 0 iff S_hi == S_lo == 0)
  a_gt_b = (V > 0), a_eq_b = (V == 0)

The device clamps V to {-1,0,1} (exact: V is integer-valued and sign-exact)
and stores a single int8 map in partition-major layout with ONE line-rate DMA
per pass; the host unpermutes and decodes the two boolean maps. That cuts
output HBM traffic 8x vs storing two fp32 maps and keeps the HWDGE input
queues free of small store descriptors.

Sharding: data parallel along dim 0 across 8 cores. Rows per core padded to
a multiple of 128 (250112 = 128 * 1954); only the last core's shard needs
host-side zero padding (896 rows), and the pad rows are dropped on gather.

Input layout options (host-side, same bytes and dtype either way):
  sep: A and B as separate DRAM tensors, streamed on the two HWDGE rings.
  ab:  A and B tiles interleaved into one DRAM tensor so HBM sees a single
       sequential address stream; one DMA per tile, alternating rings.
"""

import numpy as np

N = 2_000_000
BITS = 32
NCORES = 8
P = 128
ROWS_PER_CORE = 250_112          # 128 * 1954 >= 2_000_000 / 8
GROUPS = ROWS_PER_CORE // P      # 1954 rows per partition per core
K_MAIN = 128
KS = [K_MAIN] * (GROUPS // K_MAIN) + ([GROUPS % K_MAIN] if GROUPS % K_MAIN else [])
SEG = 16                         # bits per exact-sum segment
IO_BUFS = 4
LAYOUT = "sep"                   # "sep" or "ab"
STORE = "pass"                   # "tile": per-tile stores; "pass": one store/pass
OUT_INT8 = True                  # with STORE="pass": clamp V to {-1,0,1} int8
INPUT_CAST = True                # SWDGE fp32->bf16 cast during input DMA

_CACHE = {}


def _weight_row():
    # w_i = 2^(31-i), MSB first; exact in fp32 and bf16 (powers of two).
    return (2.0 ** (31 - np.arange(BITS, dtype=np.float64))).astype(np.float32)


def _compute_tail(nc, pool, spool, wt, ov_flat, mybir, row0, k, d,
                  vb=None, g0=0):
    """Shared mult/reduce/add/store chain for one tile, given d = a - b."""
    dt = mybir.dt
    Alu = mybir.AluOpType
    Axis = mybir.AxisListType
    rows = P * k
    F = k * BITS

    nc.vector.tensor_tensor(d[:], d[:], wt[:, :F], Alu.mult)

    # Segmented sums of 16 -> [P, 2k] (hi, lo interleaved per row);
    # accumulation is fp32 (out dtype), every addend exact.
    s = spool.tile([P, 2 * k], dt.float32, tag="s")
    nc.vector.tensor_reduce(
        out=s[:],
        in_=d[:].rearrange("p (g x) -> p g x", x=SEG),
        axis=Axis.X,
        op=Alu.add,
    )

    s3 = s[:].rearrange("p (r two) -> p r two", two=2)
    if vb is None:
        # V = S_hi + S_lo, emitted directly as bf16 for a compact store.
        v = spool.tile([P, k], dt.bfloat16, tag="v")
        nc.vector.tensor_tensor(v[:], s3[:, :, 0:1], s3[:, :, 1:2], Alu.add)
        # Tiny store goes on gpsimd SWDGE, keeping HWDGE queues for inputs.
        nc.gpsimd.dma_start(
            out=ov_flat[row0:row0 + rows].rearrange("(p r) -> p r", p=P),
            in_=v[:])
    elif OUT_INT8:
        # Clamp V to {-1,0,1} (exact: V is integer-valued) and pack int8
        # into the pass-level buffer; one line-rate store per pass.
        v32 = spool.tile([P, k], dt.float32, tag="v32")
        nc.vector.tensor_tensor(v32[:], s3[:, :, 0:1], s3[:, :, 1:2], Alu.add)
        nc.vector.tensor_scalar(v32[:], v32[:], 1.0, None, Alu.min)
        nc.vector.tensor_scalar(vb[:, g0:g0 + k], v32[:], -1.0, None, Alu.max)
    else:
        nc.vector.tensor_tensor(vb[:, g0:g0 + k], s3[:, :, 0:1], s3[:, :, 1:2],
                                Alu.add)


def _emit_pass_sep(nc, pool, spool, wt, a_flat, b_flat, ov_flat, mybir,
                   dma_only=False, vpool=None, ov2d=None):
    dt = mybir.dt
    Alu = mybir.AluOpType
    vb = None
    if STORE == "pass" and not dma_only:
        odt = dt.int8 if OUT_INT8 else dt.bfloat16
        vb = vpool.tile([P, GROUPS], odt, tag="vb")
    row0 = 0
    g0 = 0
    in_dt = dt.bfloat16 if INPUT_CAST else dt.float32
    for k in KS:
        rows = P * k
        F = k * BITS
        a = pool.tile([P, F], in_dt, tag="a")
        b = pool.tile([P, F], in_dt, tag="b")
        av = a_flat[row0 * BITS:(row0 + rows) * BITS].rearrange("(p f) -> p f", p=P)
        bv = b_flat[row0 * BITS:(row0 + rows) * BITS].rearrange("(p f) -> p f", p=P)
        if INPUT_CAST:
            # SWDGE casts fp32->bf16 in the SDMA datapath: HBM still reads
            # the full fp32 stream, but only half the bytes cross the SBUF
            # AXI ports ({0,1} is exact in bf16).
            nc.gpsimd.dma_start(out=a[:], in_=av)
            nc.gpsimd.dma_start(out=b[:], in_=bv)
        else:
            # Split input streaming across both HWDGE issuing engines.
            nc.sync.dma_start(out=a[:], in_=av)
            nc.scalar.dma_start(out=b[:], in_=bv)
        if not dma_only:
            # d <- (a - b) in bf16 (exact: values in {-1,0,1}); frees a/b
            # after one op, and bf16 runs the mult/reduce at 2x DVE rate.
            d = pool.tile([P, F], dt.bfloat16, tag="d")
            nc.vector.tensor_tensor(d[:], a[:], b[:], Alu.subtract)
            _compute_tail(nc, pool, spool, wt, ov_flat, mybir, row0, k, d,
                          vb=vb, g0=g0)
        row0 += rows
        g0 += k
    assert row0 == ROWS_PER_CORE
    if vb is not None:
        # Plain int8 copy: use the otherwise-idle HWDGE ring so the store
        # never interleaves with the SWDGE input queue.
        (nc.sync if INPUT_CAST else nc.gpsimd).dma_start(
            out=ov2d[:, :], in_=vb[:])


def _emit_pass_ab(nc, pool, spool, wt, ab_flat, ov_flat, mybir,
                  dma_only=False):
    dt = mybir.dt
    Alu = mybir.AluOpType
    row0 = 0
    for ti, k in enumerate(KS):
        rows = P * k
        F = k * BITS
        t = pool.tile([P, 2 * F], dt.float32, tag="t")
        tv = ab_flat[row0 * 2 * BITS:(row0 + rows) * 2 * BITS] \
            .rearrange("(p f) -> p f", p=P)
        # One sequential 2F-wide stream per tile; alternate HWDGE rings.
        eng = nc.sync if ti % 2 == 0 else nc.scalar
        eng.dma_start(out=t[:], in_=tv)
        if not dma_only:
            d = pool.tile([P, F], dt.bfloat16, tag="d")
            nc.vector.tensor_tensor(d[:], t[:, :F], t[:, F:], Alu.subtract)
            _compute_tail(nc, pool, spool, wt, ov_flat, mybir, row0, k, d)
        row0 += rows
    assert row0 == ROWS_PER_CORE


def _legalize_waits(nc, mybir):
    """TRN2 ISA structs accept at most one sync wait per instruction (walrus
    codegen hard-errors otherwise). Tile's scheduler attaches one wait per
    dependency, so hoist all-but-one wait onto same-engine NoOps inserted
    immediately before; engines execute in order, so semantics are identical."""
    for fn in nc.m.functions:
        for blk in fn.blocks:
            new_insts = []
            for inst in blk.instructions:
                si = inst.sync_info
                waits = list(si.on_wait) if si is not None else []
                limit = 2 if isinstance(inst, mybir.InstEventSemaphore) else 1
                if len(waits) > limit:
                    for w in waits[:-limit]:
                        nop = mybir.InstNoOp(
                            name=nc.get_next_instruction_name(),
                            sync_info=mybir.SyncInfo(on_wait=[w], on_update=[]),
                            bass_nofuse=True,
                            engine=inst.engine,
                        )
                        nc.register_instruction(nop)
                        new_insts.append(nop)
                    si.on_wait = waits[-limit:]
                new_insts.append(inst)
            blk.instructions[:] = new_insts


def _build_program(repeat=1, dma_only=False):
    key = ("nc", repeat, dma_only, LAYOUT, K_MAIN, IO_BUFS, tuple(KS),
           STORE, OUT_INT8, INPUT_CAST)
    if key in _CACHE:
        return _CACHE[key]

    from concourse.bass import Bass
    from concourse.tile import TileContext
    import concourse.mybir as mybir

    dt = mybir.dt

    nc = Bass(name="cmp32")
    if LAYOUT == "sep":
        A = nc.dram_tensor("A", [ROWS_PER_CORE, BITS], dt.float32,
                           kind="ExternalInput")
        B = nc.dram_tensor("B", [ROWS_PER_CORE, BITS], dt.float32,
                           kind="ExternalInput")
        in_flats = (A[:].flatten(), B[:].flatten())
    else:
        AB = nc.dram_tensor("AB", [2 * ROWS_PER_CORE, BITS], dt.float32,
                            kind="ExternalInput")
        in_flats = (AB[:].flatten(),)
    W = nc.dram_tensor("W", [P, K_MAIN * BITS], dt.bfloat16, kind="ExternalInput")
    if STORE == "pass":
        odt = dt.int8 if OUT_INT8 else dt.bfloat16
        OV = nc.dram_tensor("OV", [P, GROUPS], odt, kind="ExternalOutput")
        ov_flat, ov2d = None, OV
    else:
        OV = nc.dram_tensor("OV", [ROWS_PER_CORE, 1], dt.bfloat16,
                            kind="ExternalOutput")
        ov_flat, ov2d = OV[:].flatten(), None

    with TileContext(nc) as tc:
        with tc.tile_pool(name="wpool", bufs=1) as wpool, \
             tc.tile_pool(name="io", bufs=IO_BUFS) as pool, \
             tc.tile_pool(name="small", bufs=4) as spool, \
             tc.tile_pool(name="vpass", bufs=2) as vpool:
            # bf16 weight tile (weights are powers of two: exact in bf16)
            wt = wpool.tile([P, K_MAIN * BITS], dt.bfloat16)
            nc.gpsimd.dma_start(out=wt[:], in_=W[:])

            for _rep in range(repeat):
                if LAYOUT == "sep":
                    _emit_pass_sep(nc, pool, spool, wt, *in_flats, ov_flat,
                                   mybir, dma_only=dma_only,
                                   vpool=vpool, ov2d=ov2d)
                else:
                    _emit_pass_ab(nc, pool, spool, wt, *in_flats, ov_flat,
                                  mybir, dma_only=dma_only)

    _legalize_waits(nc, mybir)
    _CACHE[key] = nc
    return nc


def _interleave_ab(a_sh, b_sh):
    """Per-core: tile-interleaved single stream. For each tile of k row-groups,
    partition p's k a-rows are followed by its k b-rows; tiles are sequential.
    Same bytes and dtype as the inputs, only the address layout changes."""
    out = np.empty((2 * ROWS_PER_CORE, BITS), dtype=np.float32)
    o = 0
    row0 = 0
    for k in KS:
        rows = P * k
        blk_a = a_sh[row0:row0 + rows].reshape(P, k * BITS)
        blk_b = b_sh[row0:row0 + rows].reshape(P, k * BITS)
        blk = np.concatenate([blk_a, blk_b], axis=1)   # [P, 2F]
        out[o:o + 2 * rows] = blk.reshape(2 * rows, BITS)
        o += 2 * rows
        row0 += rows
    return out


def _shard_inputs(A, B):
    """Split full inputs into 8 per-core maps (zero-pad only the last core)."""
    import ml_dtypes
    w_tile = np.tile(_weight_row(), (P, K_MAIN)).astype(ml_dtypes.bfloat16)
    total = ROWS_PER_CORE * NCORES
    pad = total - N
    in_maps = []
    for c in range(NCORES):
        lo, hi = c * ROWS_PER_CORE, (c + 1) * ROWS_PER_CORE
        if hi <= N:
            a_sh, b_sh = A[lo:hi], B[lo:hi]
        else:
            z = np.zeros((pad, BITS), dtype=np.float32)
            a_sh = np.concatenate([A[lo:N], z])
            b_sh = np.concatenate([B[lo:N], z])
        if LAYOUT == "sep":
            in_maps.append({"A": a_sh, "B": b_sh, "W": w_tile})
        else:
            in_maps.append({"AB": _interleave_ab(a_sh, b_sh), "W": w_tile})
    return in_maps


def _decode(v):
    """Device output V (bf16) -> the two fp32 boolean maps."""
    v32 = np.asarray(v).astype(np.float32)
    og = (v32 > 0).astype(np.float32)
    oe = (v32 == 0).astype(np.float32)
    return og, oe


def _core_v_from_pass(arr):
    """Unpermute one core's [P, GROUPS] pass-store output to row order."""
    out = np.empty((ROWS_PER_CORE,), dtype=np.float32)
    g0 = row0 = 0
    for k in KS:
        out[row0:row0 + P * k] = arr[:, g0:g0 + k].astype(np.float32).reshape(-1)
        g0 += k
        row0 += P * k
    return out


def _postprocess(core_arrs):
    """Per-core device outputs -> (a_gt_b, a_eq_b) full fp32 maps."""
    if STORE == "pass":
        v32 = np.concatenate([_core_v_from_pass(np.asarray(a))
                              for a in core_arrs])[:N].reshape(-1, 1)
        og = (v32 > 0).astype(np.float32)
        oe = (v32 == 0).astype(np.float32)
        return og, oe
    v = np.concatenate([np.asarray(a) for a in core_arrs])[:N]
    return _decode(v)


def kernel(A, B):
    from concourse.bass_utils import run_bass_kernel_spmd

    A = np.ascontiguousarray(A, dtype=np.float32)
    B = np.ascontiguousarray(B, dtype=np.float32)
    assert A.shape == (N, BITS) and B.shape == (N, BITS)

    nc = _build_program()
    in_maps = _shard_inputs(A, B)
    res = run_bass_kernel_spmd(nc, in_maps, core_ids=list(range(NCORES)))
    og, oe = _postprocess([r["OV"] for r in res.results])
    return og, oe
